# revision 42
# baseline (speedup 1.0000x reference)
# Trainium2 Bass kernel for the EmbodiedCTRNN problem.
#
# Model (reference semantics):
#   x_proj = einsum("tbi,hi->tbh", x, W_i2h) + b_i2h
#   step t: pre   = x_proj[t] + h @ W_h2h.T + b_h2h + b @ W_b2h.T + b_b2h
#           h_new = h*(1-a) + relu(pre)*a
#           b_new = b*mask + a*(h @ W_h2b.T + b_h2b)
#   outputs: hidden_out [T,B,H], body_out [T,B,BODY], h_fin [B,H]
#
# Strategy: data-parallel over batch (B=256 -> 8 cores x 32). Per core the
# state is kept transposed in SBUF ([H-on-partitions, batch-in-free]) so the
# per-step vector/scalar ops are 128-partition wide and cheap. The input
# projection GEMM runs on-device in 8-step blocks (fp32r, N=256 moving)
# directly into PSUM; the recurrence accumulates h2h/b2h on the PE per
# 128-row output chunk and adds the projection from PSUM. The combined
# hidden bias rides a ones-row appended to the body state (65-partition
# b2h stationary). Outputs are written transposed ([H, T, b]) with
# contiguous DMA runs and re-transposed on the host during unsharding.

import numpy as np

T, B, IN, H, BODY = 512, 256, 128, 512, 64
ALPHA = 0.1
NCORES = 8
BL = B // NCORES          # 32 batch per core
NCH = 4                   # H / 128 chunks
CH = 32                   # timesteps per xT DMA chunk
GB = 8                    # timesteps per x_proj GEMM block (N = GB*BL = 256)
S = 16                    # timesteps per output-staging block

RECUR_BF16 = True


def _build(t_total=T, recur_bf16=RECUR_BF16, gemm_f32r=True, j_split=False):
    import concourse.bass as bass
    import concourse.mybir as mybir
    from concourse import bacc
    from concourse.bass import ts
    from concourse.tile import TileContext

    f32 = mybir.dt.float32
    f32r = mybir.dt.float32r
    bf16 = mybir.dt.bfloat16
    wdt = bf16 if recur_bf16 else f32
    AF = mybir.ActivationFunctionType
    ALU = mybir.AluOpType

    nc = bacc.Bacc(None, target_bir_lowering=False)

    xdt = f32r if gemm_f32r else f32
    xT_d = nc.dram_tensor("xT", [IN, t_total * BL], xdt, kind="ExternalInput")
    whh_d = nc.dram_tensor("whhT", [NCH, 128, H], wdt, kind="ExternalInput")
    wi_d = nc.dram_tensor("wi2hT", [IN, H], xdt, kind="ExternalInput")
    wb_d = nc.dram_tensor("wb2hT", [BODY, H], wdt, kind="ExternalInput")
    hbias_d = nc.dram_tensor("hbias", [128, NCH], f32, kind="ExternalInput")
    whb_d = nc.dram_tensor("wh2bT", [NCH, 128, BODY], wdt, kind="ExternalInput")
    bbias_d = nc.dram_tensor("bbias", [BODY, 1], f32, kind="ExternalInput")  # alpha*b_h2b
    mask_d = nc.dram_tensor("maskT", [BODY, 1], f32, kind="ExternalInput")

    hoT_d = nc.dram_tensor("hoT", [NCH, 128, t_total, BL], f32, kind="ExternalOutput")
    boT_d = nc.dram_tensor("boT", [BODY, t_total, BL], f32, kind="ExternalOutput")

    assert t_total % CH == 0 and CH % S == 0 and S % GB == 0
    n_chunks = t_total // CH

    with TileContext(nc) as tc:
        with (
            tc.tile_pool(name="weights", bufs=1) as wpool,
            tc.tile_pool(name="xt", bufs=2) as xt_pool,
            tc.tile_pool(name="xg", bufs=2) as xg_pool,
            tc.tile_pool(name="hst", bufs=2) as hst_pool,
            tc.tile_pool(name="bst", bufs=2) as bst_pool,
            tc.tile_pool(name="rt", bufs=3) as rt_pool,
            tc.tile_pool(name="rbt", bufs=3) as rbt_pool,
            tc.tile_pool(name="psum_pre", bufs=4, space="PSUM") as pre_pool,
            tc.tile_pool(name="psum_b", bufs=2, space="PSUM") as pb_pool,
            tc.tile_pool(name="psum_g", bufs=1, space="PSUM") as pg_pool,
        ):
            # --- resident weights ---
            whh_sb = wpool.tile([128, NCH, H], wdt, tag="whh")
            for j in range(NCH):
                nc.sync.dma_start(out=whh_sb[:, j, :], in_=whh_d[j])
            wi_sb = wpool.tile([IN, H], xdt, tag="wi")
            nc.sync.dma_start(out=wi_sb[:], in_=wi_d[:])
            wb_sb = wpool.tile([BODY, H], wdt, tag="wb")
            nc.sync.dma_start(out=wb_sb[:], in_=wb_d[:])
            hbias_sb = wpool.tile([128, NCH], f32, tag="hbias")
            nc.sync.dma_start(out=hbias_sb[:], in_=hbias_d[:])
            whb_sb = wpool.tile([128, NCH, BODY], wdt, tag="whb")
            for j in range(NCH):
                nc.sync.dma_start(out=whb_sb[:, j, :], in_=whb_d[j])
            abbias_sb = wpool.tile([BODY, 1], f32, tag="abbias")
            nc.sync.dma_start(out=abbias_sb[:], in_=bbias_d[:])
            mask_sb = wpool.tile([BODY, 1], f32, tag="mask")
            nc.sync.dma_start(out=mask_sb[:], in_=mask_d[:])
            c09_sb = wpool.tile([128, NCH, BL], f32, tag="c09")
            nc.gpsimd.memset(c09_sb[:], 1.0 - ALPHA)
            maskf_sb = wpool.tile([BODY, BL], f32, tag="maskf")
            nc.gpsimd.memset(maskf_sb[:], 1.0)
            nc.vector.tensor_scalar(
                maskf_sb[:], maskf_sb[:], mask_sb[:, 0:1], None, op0=ALU.mult
            )

            prev_h = None  # last h-state slot [128, NCH, BL] (fp32)
            prev_hB = None
            prev_b = None  # last body slot [BODY+1, BL] (fp32, +ones row)
            prev_bB = None

            xt_tiles = {}
            xg_tiles = {}

            def load_xt(c):
                if c >= n_chunks or c in xt_tiles:
                    return
                xt = xt_pool.tile([IN, CH * BL], xdt, tag="xt")
                nc.sync.dma_start(
                    out=xt[:], in_=xT_d[:, c * CH * BL : (c + 1) * CH * BL]
                )
                xt_tiles[c] = xt

            def emit_gemm(gi):
                # x_proj for steps [gi*GB, (gi+1)*GB) -> PSUM -> SBUF
                if gi >= t_total // GB or gi in xg_tiles:
                    return
                xt = xt_tiles[gi * GB // CH]
                col0 = (gi * GB) % CH * BL
                pg = pg_pool.tile([128, NCH, GB, BL], f32, tag="pg")
                for m in range(NCH):
                    nc.tensor.matmul(
                        pg[:, m, :, :],
                        wi_sb[:, ts(m, 128)],
                        xt[:, col0 : col0 + GB * BL],
                    )
                xg = xg_pool.tile([128, NCH, GB, BL], f32, tag="xg")
                for m in range(NCH):
                    nc.scalar.activation(
                        xg[:, m, :, :], pg[:, m, :, :], AF.Identity,
                        bias=hbias_sb[:, m : m + 1],
                    )
                xg_tiles[gi] = xg

            load_xt(0)
            emit_gemm(0)

            hst = bst = hstB = bstB = mv_h = mv_b = None
            for t in range(t_total):
                s = t % S
                if s == 0:
                    hst = hst_pool.tile([128, NCH, S + 1, BL], f32, tag="hst")
                    bst = bst_pool.tile([BODY, S + 1, BL], f32, tag="bst")
                    if recur_bf16:
                        hstB = hst_pool.tile([128, NCH, S + 1, BL], bf16, tag="hstB")
                        bstB = bst_pool.tile([BODY, S + 1, BL], bf16, tag="bstB")
                        mv_h, mv_b = hstB, bstB
                    else:
                        mv_h, mv_b = hst, bst
                    if prev_h is None:
                        nc.vector.memset(hst[:, :, 0, :], 0.0)
                        nc.vector.memset(bst[:BODY, 0, :], 0.0)
                        if recur_bf16:
                            nc.vector.memset(hstB[:, :, 0, :], 0.0)
                            nc.vector.memset(bstB[:BODY, 0, :], 0.0)
                    else:
                        nc.vector.tensor_copy(hst[:, :, 0, :], prev_h)
                        nc.vector.tensor_copy(bst[:BODY, 0, :], prev_b)
                        if recur_bf16:
                            nc.vector.tensor_copy(hstB[:, :, 0, :], prev_hB)
                            nc.vector.tensor_copy(bstB[:BODY, 0, :], prev_bB)

                gi = t // GB
                xg = xg_tiles[gi]
                sg = t % GB

                pp0 = pre_pool.tile([128, NCH, BL], f32, tag="pre")
                pp1 = pre_pool.tile([128, NCH, BL], f32, tag="pre")
                pbt = pb_pool.tile([BODY, BL], f32, tag="pb")
                pb = pbt[:]
                mvb_s = mv_b[:, s, :]
                if j_split:
                    # early section (needs only j0..j2 states)
                    for j in range(3):
                        nc.tensor.matmul(
                            pb, whb_sb[:, j, :], mv_h[:, j, s, :],
                            start=(j == 0), stop=False,
                        )
                    for m in range(NCH):
                        pp = pp0[:, m, :] if m < 3 else pp1[:, 3, :]
                        for j in range(3):
                            nc.tensor.matmul(
                                pp, whh_sb[:, j, ts(m, 128)], mv_h[:, j, s, :],
                                start=(j == 0), stop=False,
                            )
                    # late section (needs j3 + body states)
                    nc.tensor.matmul(
                        pb, whb_sb[:, 3, :], mv_h[:, 3, s, :],
                        start=False, stop=True,
                    )
                    for m in range(NCH):
                        pp = pp0[:, m, :] if m < 3 else pp1[:, 3, :]
                        nc.tensor.matmul(
                            pp, whh_sb[:, 3, ts(m, 128)], mv_h[:, 3, s, :],
                            start=False, stop=False,
                        )
                        nc.tensor.matmul(
                            pp, wb_sb[:, ts(m, 128)], mvb_s,
                            start=False, stop=True,
                        )
                else:
                    for m in range(NCH):
                        pp = pp0[:, m, :] if m < 3 else pp1[:, 3, :]
                        for j in range(NCH):
                            nc.tensor.matmul(
                                pp, whh_sb[:, j, ts(m, 128)], mv_h[:, j, s, :],
                                start=(j == 0), stop=False,
                            )
                        nc.tensor.matmul(
                            pp, wb_sb[:, ts(m, 128)], mvb_s,
                            start=False, stop=True,
                        )
                    for j in range(NCH):
                        nc.tensor.matmul(
                            pb, whb_sb[:, j, :], mv_h[:, j, s, :],
                            start=(j == 0), stop=(j == NCH - 1),
                        )


                # --- epilogue: group 0 = chunks 0..2, group 1 = chunk 3 ---
                # Critical chain per group: TT(add x_proj) -> ACT relu ->
                # bf16 blend (DVE, feeds next step's matmuls). The fp32
                # state blend is identical math but off the critical path,
                # so it runs on the otherwise-idle GPSIMD engine.
                rt = rt_pool.tile([128, NCH, BL], f32, tag="rt")
                nc.vector.tensor_add(
                    pp0[:, 0:3, :], pp0[:, 0:3, :], xg[:, 0:3, sg, :]
                )
                nc.vector.tensor_scalar(
                    rt[:, 0:3, :], pp0[:, 0:3, :], 0.0, ALPHA,
                    op0=ALU.max, op1=ALU.mult,
                )
                nc.vector.scalar_tensor_tensor(
                    mv_h[:, 0:3, s + 1, :], hst[:, 0:3, s, :],
                    1.0 - ALPHA, rt[:, 0:3, :],
                    op0=ALU.mult, op1=ALU.add,
                )
                nc.vector.tensor_add(pp1[:, 3, :], pp1[:, 3, :], xg[:, 3, sg, :])
                nc.scalar.activation(
                    rt[:, 3, :], pp1[:, 3, :], AF.Relu, scale=ALPHA
                )
                nc.vector.scalar_tensor_tensor(
                    mv_h[:, 3, s + 1, :], hst[:, 3, s, :],
                    1.0 - ALPHA, rt[:, 3, :],
                    op0=ALU.mult, op1=ALU.add,
                )
                if recur_bf16:
                    htmp = rbt_pool.tile([128, NCH, BL], f32, tag="htmp")
                    nc.gpsimd.tensor_mul(htmp[:], hst[:, :, s, :], c09_sb[:])
                    nc.gpsimd.tensor_add(
                        hst[:, :, s + 1, :], htmp[:], rt[:, :, :]
                    )
                # body: b_new = b*mask + a*(psum_b + b_h2b)
                rbt = rbt_pool.tile([BODY, BL], f32, tag="rbt")
                nc.scalar.activation(
                    rbt[:], pb, AF.Identity,
                    bias=abbias_sb[:, 0:1], scale=ALPHA,
                )
                nc.vector.scalar_tensor_tensor(
                    mv_b[:, s + 1, :], bst[:BODY, s, :],
                    mask_sb[:, 0:1], rbt[:],
                    op0=ALU.mult, op1=ALU.add,
                )
                if recur_bf16:
                    btmp = rbt_pool.tile([BODY, BL], f32, tag="btmp")
                    nc.gpsimd.tensor_mul(btmp[:], bst[:BODY, s, :], maskf_sb[:])
                    nc.gpsimd.tensor_add(bst[:BODY, s + 1, :], btmp[:], rbt[:])

                if sg == GB - 2:
                    if t % CH == CH - 2 and t // CH + 1 < n_chunks:
                        load_xt(t // CH + 1)
                    emit_gemm(gi + 1)

                if sg == GB - 1:
                    xg_tiles.pop(gi)
                    if t % CH == CH - 1:
                        xt_tiles.pop(t // CH)

                if s == S - 1:
                    # --- stage out this block ---
                    tg = t - S + 1
                    for m in range(NCH):
                        nc.sync.dma_start(
                            out=hoT_d[m, :, tg : tg + S, :],
                            in_=hst[:, m, 1 : S + 1, :],
                        )
                    nc.sync.dma_start(
                        out=boT_d[:, tg : tg + S, :], in_=bst[:BODY, 1 : S + 1, :]
                    )
                    prev_h = hst[:, :, S, :]
                    prev_b = bst[:BODY, S, :]
                    if recur_bf16:
                        prev_hB = hstB[:, :, S, :]
                        prev_bB = bstB[:BODY, S, :]

    nc.finalize()
    return nc


def _prep_inputs(inputs, t_total=T, recur_bf16=RECUR_BF16):
    import ml_dtypes

    wdt = ml_dtypes.bfloat16 if recur_bf16 else np.float32
    x = np.asarray(inputs["x"], np.float32)[:t_total]
    W_i2h = np.asarray(inputs["W_i2h"], np.float32)
    b_i2h = np.asarray(inputs["b_i2h"], np.float32)
    W_h2h = np.asarray(inputs["W_h2h"], np.float32)
    b_h2h = np.asarray(inputs["b_h2h"], np.float32)
    W_b2h = np.asarray(inputs["W_b2h"], np.float32)
    b_b2h = np.asarray(inputs["b_b2h"], np.float32)
    W_h2b = np.asarray(inputs["W_h2b"], np.float32)
    b_h2b = np.asarray(inputs["b_h2b"], np.float32)
    mask = np.asarray(inputs["body_mask"], np.float32)

    hbias = b_i2h + b_h2h + b_b2h  # added into x_proj blocks on-device
    wb2hT = W_b2h.T
    shared = {
        "whhT": np.ascontiguousarray(W_h2h.T.reshape(NCH, 128, H).astype(wdt)),
        "wi2hT": np.ascontiguousarray(W_i2h.T),
        "wb2hT": np.ascontiguousarray(wb2hT.astype(wdt)),
        "wh2bT": np.ascontiguousarray(W_h2b.T.reshape(NCH, 128, BODY).astype(wdt)),
        "bbias": np.ascontiguousarray((ALPHA * b_h2b).reshape(BODY, 1)),
        "maskT": np.ascontiguousarray(mask.reshape(BODY, 1)),
    }
    shared["hbias"] = np.ascontiguousarray(hbias.reshape(NCH, 128).T)
    in_maps = []
    for c in range(NCORES):
        xc = x[:, c * BL : (c + 1) * BL, :]  # [T, BL, IN]
        xT = np.ascontiguousarray(xc.transpose(2, 0, 1).reshape(IN, t_total * BL))
        in_maps.append({"xT": xT, **shared})
    return in_maps


def _assemble(results, t_total=T):
    hidden = np.empty((t_total, B, H), np.float32)
    body = np.empty((t_total, B, BODY), np.float32)
    for c, res in enumerate(results):
        hoT = res["hoT"]  # [NCH, 128, T, BL]
        boT = res["boT"]  # [BODY, T, BL]
        hidden[:, c * BL : (c + 1) * BL, :] = (
            hoT.transpose(2, 3, 0, 1).reshape(t_total, BL, H)
        )
        body[:, c * BL : (c + 1) * BL, :] = boT.transpose(1, 2, 0)
    h_fin = hidden[-1].copy()
    return hidden, body, h_fin


def kernel(**inputs):
    from concourse.bass_utils import run_bass_kernel_spmd

    nc = _build(T)
    in_maps = _prep_inputs(inputs, T)
    out = run_bass_kernel_spmd(nc, in_maps, core_ids=list(range(NCORES)))
    return _assemble(out.results, T)


# revision 43
# speedup vs baseline: 1.0454x; 1.0454x over previous
# Trainium2 Bass kernel for the EmbodiedCTRNN problem.
#
# Model (reference semantics):
#   x_proj = einsum("tbi,hi->tbh", x, W_i2h) + b_i2h
#   step t: pre   = x_proj[t] + h @ W_h2h.T + b_h2h + b @ W_b2h.T + b_b2h
#           h_new = h*(1-a) + relu(pre)*a
#           b_new = b*mask + a*(h @ W_h2b.T + b_h2b)
#   outputs: hidden_out [T,B,H], body_out [T,B,BODY], h_fin [B,H]
#
# Strategy: data-parallel over batch (B=256 -> 8 cores x 32). Per core the
# state is kept transposed in SBUF ([H-on-partitions, batch-in-free]) so the
# per-step vector/scalar ops are 128-partition wide and cheap. The input
# projection GEMM runs on-device in 8-step blocks (fp32r, N=256 moving)
# directly into PSUM; the recurrence accumulates h2h/b2h on the PE per
# 128-row output chunk and adds the projection from PSUM. The combined
# hidden bias rides a ones-row appended to the body state (65-partition
# b2h stationary). Outputs are written transposed ([H, T, b]) with
# contiguous DMA runs and re-transposed on the host during unsharding.

import numpy as np

T, B, IN, H, BODY = 512, 256, 128, 512, 64
ALPHA = 0.1
NCORES = 8
BL = B // NCORES          # 32 batch per core
NCH = 4                   # H / 128 chunks
CH = 32                   # timesteps per xT DMA chunk
GB = 8                    # timesteps per x_proj GEMM block (N = GB*BL = 256)
S = 16                    # timesteps per output-staging block

RECUR_BF16 = True


def _build(t_total=T, recur_bf16=RECUR_BF16, gemm_f32r=True, j_split=False):
    import concourse.bass as bass
    import concourse.mybir as mybir
    from concourse import bacc
    from concourse.bass import ts
    from concourse.tile import TileContext

    f32 = mybir.dt.float32
    f32r = mybir.dt.float32r
    bf16 = mybir.dt.bfloat16
    wdt = bf16 if recur_bf16 else f32
    AF = mybir.ActivationFunctionType
    ALU = mybir.AluOpType

    nc = bacc.Bacc(None, target_bir_lowering=False)

    xdt = f32r if gemm_f32r else f32
    xT_d = nc.dram_tensor("xT", [IN, t_total * BL], xdt, kind="ExternalInput")
    whh_d = nc.dram_tensor("whhT", [NCH, 128, H], wdt, kind="ExternalInput")
    wi_d = nc.dram_tensor("wi2hT", [IN, H], xdt, kind="ExternalInput")
    wb_d = nc.dram_tensor("wb2hT", [BODY, H], wdt, kind="ExternalInput")
    hbias_d = nc.dram_tensor("hbias", [128, NCH], f32, kind="ExternalInput")
    whb_d = nc.dram_tensor("wh2bT", [NCH, 128, BODY], wdt, kind="ExternalInput")
    bbias_d = nc.dram_tensor("bbias", [BODY, 1], f32, kind="ExternalInput")  # alpha*b_h2b
    mask_d = nc.dram_tensor("maskT", [BODY, 1], f32, kind="ExternalInput")

    hoT_d = nc.dram_tensor("hoT", [NCH, 128, t_total, BL], f32, kind="ExternalOutput")
    boT_d = nc.dram_tensor("boT", [BODY, t_total, BL], f32, kind="ExternalOutput")

    assert t_total % CH == 0 and CH % S == 0 and S % GB == 0
    n_chunks = t_total // CH

    with TileContext(nc) as tc:
        with (
            tc.tile_pool(name="weights", bufs=1) as wpool,
            tc.tile_pool(name="xt", bufs=2) as xt_pool,
            tc.tile_pool(name="xg", bufs=2) as xg_pool,
            tc.tile_pool(name="hst", bufs=2) as hst_pool,
            tc.tile_pool(name="bst", bufs=2) as bst_pool,
            tc.tile_pool(name="rt", bufs=3) as rt_pool,
            tc.tile_pool(name="rbt", bufs=3) as rbt_pool,
            tc.tile_pool(name="psum_pre", bufs=4, space="PSUM") as pre_pool,
            tc.tile_pool(name="psum_b", bufs=2, space="PSUM") as pb_pool,
            tc.tile_pool(name="psum_g", bufs=1, space="PSUM") as pg_pool,
        ):
            # --- resident weights ---
            whh_sb = wpool.tile([128, NCH, H], wdt, tag="whh")
            for j in range(NCH):
                nc.sync.dma_start(out=whh_sb[:, j, :], in_=whh_d[j])
            wi_sb = wpool.tile([IN, H], xdt, tag="wi")
            nc.sync.dma_start(out=wi_sb[:], in_=wi_d[:])
            wb_sb = wpool.tile([BODY, H], wdt, tag="wb")
            nc.sync.dma_start(out=wb_sb[:], in_=wb_d[:])
            hbias_sb = wpool.tile([128, NCH], f32, tag="hbias")
            nc.sync.dma_start(out=hbias_sb[:], in_=hbias_d[:])
            whb_sb = wpool.tile([128, NCH, BODY], wdt, tag="whb")
            for j in range(NCH):
                nc.sync.dma_start(out=whb_sb[:, j, :], in_=whb_d[j])
            abbias_sb = wpool.tile([BODY, 1], f32, tag="abbias")
            nc.sync.dma_start(out=abbias_sb[:], in_=bbias_d[:])
            mask_sb = wpool.tile([BODY, 1], f32, tag="mask")
            nc.sync.dma_start(out=mask_sb[:], in_=mask_d[:])
            c09_sb = wpool.tile([128, NCH, BL], f32, tag="c09")
            nc.gpsimd.memset(c09_sb[:], 1.0 - ALPHA)
            maskf_sb = wpool.tile([BODY, BL], f32, tag="maskf")
            nc.gpsimd.memset(maskf_sb[:], 1.0)
            nc.vector.tensor_scalar(
                maskf_sb[:], maskf_sb[:], mask_sb[:, 0:1], None, op0=ALU.mult
            )

            prev_h = None  # last h-state slot [128, NCH, BL] (fp32)
            prev_hB = None
            prev_b = None  # last body slot [BODY+1, BL] (fp32, +ones row)
            prev_bB = None

            xt_tiles = {}
            xg_tiles = {}

            def load_xt(c):
                if c >= n_chunks or c in xt_tiles:
                    return
                xt = xt_pool.tile([IN, CH * BL], xdt, tag="xt")
                nc.sync.dma_start(
                    out=xt[:], in_=xT_d[:, c * CH * BL : (c + 1) * CH * BL]
                )
                xt_tiles[c] = xt

            def emit_gemm(gi):
                # x_proj for steps [gi*GB, (gi+1)*GB) -> PSUM -> SBUF
                if gi >= t_total // GB or gi in xg_tiles:
                    return
                xt = xt_tiles[gi * GB // CH]
                col0 = (gi * GB) % CH * BL
                pg = pg_pool.tile([128, NCH, GB, BL], f32, tag="pg")
                for m in range(NCH):
                    nc.tensor.matmul(
                        pg[:, m, :, :],
                        wi_sb[:, ts(m, 128)],
                        xt[:, col0 : col0 + GB * BL],
                    )
                xg = xg_pool.tile([128, NCH, GB, BL], f32, tag="xg")
                for m in range(NCH):
                    nc.scalar.activation(
                        xg[:, m, :, :], pg[:, m, :, :], AF.Identity,
                        bias=hbias_sb[:, m : m + 1],
                    )
                xg_tiles[gi] = xg

            load_xt(0)
            emit_gemm(0)

            hst = bst = hstB = bstB = mv_h = mv_b = None
            for t in range(t_total):
                s = t % S
                if s == 0:
                    hst = hst_pool.tile([128, NCH, S + 1, BL], f32, tag="hst")
                    bst = bst_pool.tile([BODY, S + 1, BL], f32, tag="bst")
                    if recur_bf16:
                        hstB = hst_pool.tile([128, NCH, S + 1, BL], bf16, tag="hstB")
                        bstB = bst_pool.tile([BODY, S + 1, BL], bf16, tag="bstB")
                        mv_h, mv_b = hstB, bstB
                    else:
                        mv_h, mv_b = hst, bst
                    if prev_h is None:
                        nc.vector.memset(hst[:, :, 0, :], 0.0)
                        nc.vector.memset(bst[:BODY, 0, :], 0.0)
                        if recur_bf16:
                            nc.vector.memset(hstB[:, :, 0, :], 0.0)
                            nc.vector.memset(bstB[:BODY, 0, :], 0.0)
                    else:
                        nc.vector.tensor_copy(hst[:, :, 0, :], prev_h)
                        nc.vector.tensor_copy(bst[:BODY, 0, :], prev_b)
                        if recur_bf16:
                            nc.vector.tensor_copy(hstB[:, :, 0, :], prev_hB)
                            nc.vector.tensor_copy(bstB[:BODY, 0, :], prev_bB)

                gi = t // GB
                xg = xg_tiles[gi]
                sg = t % GB

                pp0 = pre_pool.tile([128, NCH, BL], f32, tag="pre")
                pp1 = pre_pool.tile([128, NCH, BL], f32, tag="pre")
                pbt = pb_pool.tile([BODY, BL], f32, tag="pb")
                pb = pbt[:]
                mvb_s = mv_b[:, s, :]
                if j_split:
                    # early section (needs only j0..j2 states)
                    for j in range(3):
                        nc.tensor.matmul(
                            pb, whb_sb[:, j, :], mv_h[:, j, s, :],
                            start=(j == 0), stop=False,
                        )
                    for m in range(NCH):
                        pp = pp0[:, m, :] if m < 3 else pp1[:, 3, :]
                        for j in range(3):
                            nc.tensor.matmul(
                                pp, whh_sb[:, j, ts(m, 128)], mv_h[:, j, s, :],
                                start=(j == 0), stop=False,
                            )
                    # late section (needs j3 + body states)
                    nc.tensor.matmul(
                        pb, whb_sb[:, 3, :], mv_h[:, 3, s, :],
                        start=False, stop=True,
                    )
                    for m in range(NCH):
                        pp = pp0[:, m, :] if m < 3 else pp1[:, 3, :]
                        nc.tensor.matmul(
                            pp, whh_sb[:, 3, ts(m, 128)], mv_h[:, 3, s, :],
                            start=False, stop=False,
                        )
                        nc.tensor.matmul(
                            pp, wb_sb[:, ts(m, 128)], mvb_s,
                            start=False, stop=True,
                        )
                else:
                    for j in range(NCH):
                        nc.tensor.matmul(
                            pb, whb_sb[:, j, :], mv_h[:, j, s, :],
                            start=(j == 0), stop=(j == NCH - 1),
                        )
                    for m in range(NCH):
                        pp = pp0[:, m, :] if m < 3 else pp1[:, 3, :]
                        for j in range(NCH):
                            nc.tensor.matmul(
                                pp, whh_sb[:, j, ts(m, 128)], mv_h[:, j, s, :],
                                start=(j == 0), stop=False,
                            )
                        nc.tensor.matmul(
                            pp, wb_sb[:, ts(m, 128)], mvb_s,
                            start=False, stop=True,
                        )


                # --- epilogue: group 0 = chunks 0..2, group 1 = chunk 3 ---
                # Critical chain per group: TT(add x_proj) -> ACT relu ->
                # bf16 blend (DVE, feeds next step's matmuls). The fp32
                # state blend is identical math but off the critical path,
                # so it runs on the otherwise-idle GPSIMD engine.
                rt = rt_pool.tile([128, NCH, BL], f32, tag="rt")
                nc.vector.tensor_add(
                    pp0[:, 0:3, :], pp0[:, 0:3, :], xg[:, 0:3, sg, :]
                )
                nc.vector.tensor_scalar(
                    rt[:, 0:3, :], pp0[:, 0:3, :], 0.0, ALPHA,
                    op0=ALU.max, op1=ALU.mult,
                )
                nc.vector.scalar_tensor_tensor(
                    mv_h[:, 0:3, s + 1, :], hst[:, 0:3, s, :],
                    1.0 - ALPHA, rt[:, 0:3, :],
                    op0=ALU.mult, op1=ALU.add,
                )
                nc.vector.tensor_add(pp1[:, 3, :], pp1[:, 3, :], xg[:, 3, sg, :])
                nc.scalar.activation(
                    rt[:, 3, :], pp1[:, 3, :], AF.Relu, scale=ALPHA
                )
                nc.vector.scalar_tensor_tensor(
                    mv_h[:, 3, s + 1, :], hst[:, 3, s, :],
                    1.0 - ALPHA, rt[:, 3, :],
                    op0=ALU.mult, op1=ALU.add,
                )
                if recur_bf16:
                    htmp = rbt_pool.tile([128, NCH, BL], f32, tag="htmp")
                    nc.gpsimd.tensor_mul(htmp[:], hst[:, :, s, :], c09_sb[:])
                    nc.gpsimd.tensor_add(
                        hst[:, :, s + 1, :], htmp[:], rt[:, :, :]
                    )
                # body: b_new = b*mask + a*(psum_b + b_h2b)
                rbt = rbt_pool.tile([BODY, BL], f32, tag="rbt")
                nc.scalar.activation(
                    rbt[:], pb, AF.Identity,
                    bias=abbias_sb[:, 0:1], scale=ALPHA,
                )
                nc.vector.scalar_tensor_tensor(
                    mv_b[:, s + 1, :], bst[:BODY, s, :],
                    mask_sb[:, 0:1], rbt[:],
                    op0=ALU.mult, op1=ALU.add,
                )
                if recur_bf16:
                    btmp = rbt_pool.tile([BODY, BL], f32, tag="btmp")
                    nc.gpsimd.tensor_mul(btmp[:], bst[:BODY, s, :], maskf_sb[:])
                    nc.gpsimd.tensor_add(bst[:BODY, s + 1, :], btmp[:], rbt[:])

                if sg == GB - 2:
                    if t % CH == CH - 2 and t // CH + 1 < n_chunks:
                        load_xt(t // CH + 1)
                    emit_gemm(gi + 1)

                if sg == GB - 1:
                    xg_tiles.pop(gi)
                    if t % CH == CH - 1:
                        xt_tiles.pop(t // CH)

                if s == S - 1:
                    # --- stage out this block ---
                    tg = t - S + 1
                    for m in range(NCH):
                        nc.sync.dma_start(
                            out=hoT_d[m, :, tg : tg + S, :],
                            in_=hst[:, m, 1 : S + 1, :],
                        )
                    nc.sync.dma_start(
                        out=boT_d[:, tg : tg + S, :], in_=bst[:BODY, 1 : S + 1, :]
                    )
                    prev_h = hst[:, :, S, :]
                    prev_b = bst[:BODY, S, :]
                    if recur_bf16:
                        prev_hB = hstB[:, :, S, :]
                        prev_bB = bstB[:BODY, S, :]

    nc.finalize()
    return nc


def _prep_inputs(inputs, t_total=T, recur_bf16=RECUR_BF16):
    import ml_dtypes

    wdt = ml_dtypes.bfloat16 if recur_bf16 else np.float32
    x = np.asarray(inputs["x"], np.float32)[:t_total]
    W_i2h = np.asarray(inputs["W_i2h"], np.float32)
    b_i2h = np.asarray(inputs["b_i2h"], np.float32)
    W_h2h = np.asarray(inputs["W_h2h"], np.float32)
    b_h2h = np.asarray(inputs["b_h2h"], np.float32)
    W_b2h = np.asarray(inputs["W_b2h"], np.float32)
    b_b2h = np.asarray(inputs["b_b2h"], np.float32)
    W_h2b = np.asarray(inputs["W_h2b"], np.float32)
    b_h2b = np.asarray(inputs["b_h2b"], np.float32)
    mask = np.asarray(inputs["body_mask"], np.float32)

    hbias = b_i2h + b_h2h + b_b2h  # added into x_proj blocks on-device
    wb2hT = W_b2h.T
    shared = {
        "whhT": np.ascontiguousarray(W_h2h.T.reshape(NCH, 128, H).astype(wdt)),
        "wi2hT": np.ascontiguousarray(W_i2h.T),
        "wb2hT": np.ascontiguousarray(wb2hT.astype(wdt)),
        "wh2bT": np.ascontiguousarray(W_h2b.T.reshape(NCH, 128, BODY).astype(wdt)),
        "bbias": np.ascontiguousarray((ALPHA * b_h2b).reshape(BODY, 1)),
        "maskT": np.ascontiguousarray(mask.reshape(BODY, 1)),
    }
    shared["hbias"] = np.ascontiguousarray(hbias.reshape(NCH, 128).T)
    in_maps = []
    for c in range(NCORES):
        xc = x[:, c * BL : (c + 1) * BL, :]  # [T, BL, IN]
        xT = np.ascontiguousarray(xc.transpose(2, 0, 1).reshape(IN, t_total * BL))
        in_maps.append({"xT": xT, **shared})
    return in_maps


def _assemble(results, t_total=T):
    hidden = np.empty((t_total, B, H), np.float32)
    body = np.empty((t_total, B, BODY), np.float32)
    for c, res in enumerate(results):
        hoT = res["hoT"]  # [NCH, 128, T, BL]
        boT = res["boT"]  # [BODY, T, BL]
        hidden[:, c * BL : (c + 1) * BL, :] = (
            hoT.transpose(2, 3, 0, 1).reshape(t_total, BL, H)
        )
        body[:, c * BL : (c + 1) * BL, :] = boT.transpose(1, 2, 0)
    h_fin = hidden[-1].copy()
    return hidden, body, h_fin


def kernel(**inputs):
    from concourse.bass_utils import run_bass_kernel_spmd

    nc = _build(T)
    in_maps = _prep_inputs(inputs, T)
    out = run_bass_kernel_spmd(nc, in_maps, core_ids=list(range(NCORES)))
    return _assemble(out.results, T)
